# revision 11
# baseline (speedup 1.0000x reference)
"""Trainium2 Bass kernel for nn_ChannelLatencySeq2Value (B=8, C=256, T=4096).

Structure (see kernel_baseline.py for the full derivation):
  * encoder collapses to drive[b,c,t] = sum_{j<3,k<9} g[c,j,k] x[b,(3c+j)%256,t+k-4]
  * fp8(e4m3) DoubleRow matmuls pair the taps: per (time-chunk, window):
    4 DR matmuls (taps 0-7, rhs pair dim = stride-1 overlapping shifts) +
    1 plain fp8 matmul (tap 8) + 1 straddler matmul (K=36, pre-shifted rows).
  * LIF scan V = a*V + (1-a)*drive via tensor_tensor_scan in bf16 (2x DVE),
    drive evacuated PSUM->SBUF-bf16 by ScalarE.
  * per-chunk running-max via tensor_reduce (bf16 in), final reduce + DMA out.
  * 9 junk warm-up matmuls at the head keep the PE HAM at 2.4 GHz before the
    real stream arrives (the first real MM lands pre-warmed).
  * first-spike latency from per-row Vmax; rows with Vmax >= 0.90 recomputed
    exactly on the host (reference input peaks at ~0.76, fp8 device error
    <= ~0.03, so the fallback never triggers but guarantees exactness).
  * tiny (B,C) MLP head on the host in fp32.
"""

import numpy as np
import ml_dtypes

import concourse.bass as bass
import concourse.bacc as bacc
import concourse.mybir as mybir
from concourse.tile import TileContext
from concourse.ap import AP
from concourse.bass_utils import run_bass_kernel_spmd


def _ensure_axon_hooks():
    try:
        import antenv.axon_hooks  # noqa: F401
    except ImportError:
        import sys
        import types
        m = types.ModuleType("antenv.axon_hooks")
        m.get_axon_ntff_profile_hook = lambda: None
        m.set_axon_ntff_profile_hook = lambda h: None
        sys.modules["antenv.axon_hooks"] = m


_ensure_axon_hooks()

# ---------------------------------------------------------------- constants
B, C, T = 8, 256, 4096
OP = 6
ALPHA = float(np.exp(-1.0 / 5.0))
OMA = 1.0 - ALPHA
THRESHOLD = 1.0
TC = 512                      # time chunk (= one PSUM bank of fp32)
NT = T // TC
PAD = 4                       # conv halo (kernel width 9)
PADT = T + 2 * PAD
NCORES = 8
FALLBACK_THR = 0.90           # host exact-recompute margin for Vmax
NWARM = 6                     # junk matmuls that pre-warm the PE HAM clock

F8 = ml_dtypes.float8_e4m3    # == concourse float8e4 (TRN FP8_EXP4)

# mega-blob layout (fp8 bytes per partition, 128 partitions)
# AA: per window: 4 DoubleRow pair slabs (256 B: tap 2P | tap 2P+1) + tap-8
# plain slab (128 B) = 1152 B
AAW = 2 * 1152                 # [0, 2304)
XB0 = AAW                      # x ci0 piece0 (xpad cols [0,1546))
XB1 = XB0 + 1546               # x ci1 piece0
XB2 = XB1 + 1546               # x ci0 piece1 (xpad cols [1528,4104))
XB3 = XB2 + 2576               # x ci1 piece1
BLOBW = XB3 + 2576

# channel -> psum-tile assignment (see kernel_baseline.py)
PERM1 = list(range(0, 42)) + list(range(86, 128)) + list(range(171, 213)) + [42, 85]
PERM2 = list(range(43, 85)) + list(range(128, 170)) + list(range(214, 256)) + [170, 213]
W3ROWS = (0, 127, 128, 255)
STRAD = (42, 85, 170, 213)     # straddler channels: exact host path
LAM = 60.0                     # exp-sum smoothmax sharpness


def _compose_g(w3, b3, w5, b5, w9, b9, w_red, b_red):
    """Collapse the 4-conv encoder into g[c,3,9] (fp64 accum) + beta[c]."""
    g = np.zeros((C, 3, 9), np.float64)
    beta = np.zeros((C,), np.float64)
    paths = [(np.asarray(w3, np.float64), np.asarray(b3, np.float64), 3),
             (np.asarray(w5, np.float64), np.asarray(b5, np.float64), 5),
             (np.asarray(w9, np.float64), np.asarray(b9, np.float64), 9)]
    wr = np.asarray(w_red, np.float64)
    for c in range(C):
        beta[c] += float(b_red[c])
        for i in range(18):
            m = c * 18 + i
            wp, bp, K = paths[m // (C * OP)]
            q = m % (C * OP)
            s = q // OP
            j = (s - 3 * c) % 256
            assert j in (0, 1, 2)
            pad = (K - 1) // 2
            w = wr[c, i, 0]
            beta[c] += w * bp[q]
            g[c, j, 4 - pad:4 + pad + 1] += w * wp[q, 0, :]
    return g, beta


def _build_weights(g):
    """Split (1-a)*g into window lhsT stacks A1/A2 (9,128,128) and straddler
    lhsT B1/B2 (36,128)."""
    gs = g * OMA
    A = [np.zeros((9, 128, 128), np.float64) for _ in range(2)]
    Bm = [np.zeros((36, 128), np.float64) for _ in range(2)]
    for ti, perm in enumerate((PERM1, PERM2)):
        lo = 128 * ti
        for p, c in enumerate(perm):
            for j in range(3):
                s = (3 * c + j) % 256
                if lo <= s < lo + 128:
                    A[ti][:, s - lo, p] = gs[c, j, :]
                else:
                    r = W3ROWS.index(s)
                    Bm[ti][9 * r:9 * r + 9, p] = gs[c, j, :]
    return A[0], A[1], Bm[0], Bm[1]


# ------------------------------------------------------------ device program
_PROG = None
LAST_RESULTS = None
LAST_VMAX = None


def _build_program():
    f32 = mybir.dt.float32
    bf = mybir.dt.bfloat16
    f8 = mybir.dt.float8e4
    DR = mybir.MatmulPerfMode.DoubleRow
    nc = bacc.Bacc(None, target_bir_lowering=False)
    blob_d = nc.declare_dram_parameter("blob", [128, BLOBW], f8, isOutput=False)
    esum_d = nc.declare_dram_parameter("esum", [128, 10], f32, isOutput=True)

    with TileContext(nc) as tc:
        with (
            tc.tile_pool(name="cst", bufs=1) as cst,
            tc.tile_pool(name="ps", bufs=6, space="PSUM") as pp,
            tc.tile_pool(name="pw", bufs=1, space="PSUM") as pw,
            tc.tile_pool(name="dp", bufs=3) as dp,
        ):
            mt = cst.tile([128, BLOBW], f8, tag="mt")
            alpha_t = cst.tile([128, TC], bf, tag="alpha")
            junk = cst.tile([128, TC], bf, tag="junk")
            vb1 = cst.tile([128, T], bf, tag="vb1")
            vb2 = cst.tile([128, T], bf, tag="vb2")
            esc = cst.tile([128, 2 * TC], bf, tag="esc")
            esum_t = cst.tile([128, 10], f32, tag="esum")

            nc.gpsimd.memset(junk[:], 0.0)
            nc.vector.memset(alpha_t[:], ALPHA)

            # PE pre-warm: junk matmuls issue as soon as the memset lands and
            # keep the PE busy through the HAM SHORT window, so the real
            # stream below runs at 2.4 GHz early.
            wps = pw.tile([128, TC], f32, tag="warm")
            for _ in range(NWARM):
                nc.tensor.matmul(wps[:], junk[:, 0:128], junk[:],
                                 start=True, stop=True, skip_group_check=True)

            # loads via SWDGE (gpsimd), in consumption-priority order so the
            # head pieces never share bandwidth with the bulk.
            pieces = [(0, 512, 128),              # ci0 DR slabs P0-P1
                      (XB0, XB0 + 520, 128),      # x ci0 chunk0
                      (512, AAW, 128),            # weight slabs rest
                      (XB1, XB1 + 520, 128),      # x ci1 chunk0
                      (XB0 + 520, XB1, 128),      # x ci0 p0 rest
                      (XB1 + 520, XB2, 128),      # x ci1 p0 rest
                      (XB2, BLOBW, 128)]          # x p1 halves (both ci)
            for lo, hi, np_ in pieces:
                nc.gpsimd.dma_start(out=mt[0:np_, lo:hi], in_=blob_d[0:np_, lo:hi])

            mtt = mt[:].tensor

            # encoder matmuls + LIF scan + running max
            for l in range(NT):
                t0 = l * TC
                for ci, vb in enumerate((vb1, vb2)):
                    if l <= 2:
                        xoff = (XB0, XB1)[ci] + t0
                    else:
                        xoff = (XB2, XB3)[ci] + t0 - 1528
                    aoff = ci * 1152
                    ps = pp.tile([128, TC], f32, tag="ps")
                    for P in range(4):
                        lhsT = AP(mtt, aoff + P * 256,
                                  [[BLOBW, 128], [128, 2], [1, 128]])
                        rhs = AP(mtt, xoff + 2 * P,
                                 [[BLOBW, 128], [1, 2], [1, TC]])
                        nc.tensor.matmul(ps[:], lhsT, rhs, start=(P == 0),
                                         stop=False, perf_mode=DR)
                    nc.tensor.matmul(
                        ps[:], mt[:, aoff + 1024:aoff + 1152],
                        mt[:, xoff + 8:xoff + 8 + TC],
                        start=False, stop=True,
                    )
                    # the scan reads the PSUM bank directly (no evacuation
                    # copy -- ScalarE only runs the exp-sum below)
                    init = 0.0 if l == 0 else vb[:, t0 - 1:t0]
                    nc.vector.tensor_tensor_scan(
                        vb[:, t0:t0 + TC], alpha_t[:], ps[:], init,
                        mybir.AluOpType.mult, mybir.AluOpType.add,
                    )
                    # Vmax upper bound via exp-sum on the otherwise-idle
                    # ScalarE: esum[p] = sum_t exp(LAM*V) over 2-chunk pairs;
                    # host takes log(sum)/LAM >= Vmax.  The final pair is
                    # split per-chunk so the last exp only covers one chunk
                    # (shorter critical tail); its two halves go to separate
                    # esum columns summed on the host.
                    col = 4 * ci + l // 2
                    if l == NT - 2:
                        nc.scalar.activation(
                            out=esc[:, 0:TC], in_=vb[:, t0:t0 + TC],
                            func=mybir.ActivationFunctionType.Exp, scale=LAM,
                            accum_out=esum_t[:, col:col + 1],
                        )
                    elif l == NT - 1:
                        nc.scalar.activation(
                            out=esc[:, TC:2 * TC], in_=vb[:, t0:t0 + TC],
                            func=mybir.ActivationFunctionType.Exp, scale=LAM,
                            accum_out=esum_t[:, 8 + ci:8 + ci + 1],
                        )
                    elif l % 2 == 1:
                        nc.scalar.activation(
                            out=esc[:], in_=vb[:, t0 - TC:t0 + TC],
                            func=mybir.ActivationFunctionType.Exp, scale=LAM,
                            accum_out=esum_t[:, col:col + 1],
                        )
            nc.gpsimd.dma_start(out=esum_d[:], in_=esum_t[:])
    nc.compile()
    return nc


def _get_program():
    global _PROG
    if _PROG is None:
        _PROG = _build_program()
    return _PROG


# ------------------------------------------------------- host-side fallback
def _exact_row(x_row3, g_row, beta_c):
    """Exact fp32 drive + sequential LIF scan + first crossing for one (b,c)."""
    xp = np.pad(x_row3.astype(np.float32), ((0, 0), (PAD, PAD)))
    d = np.full((T,), np.float32(beta_c), np.float32)
    for j in range(3):
        for k in range(9):
            d += np.float32(g_row[j, k]) * xp[j, k:k + T]
    a = np.float32(ALPHA)
    oma = np.float32(OMA)
    V = np.float32(0.0)
    first = -1
    for t in range(T):
        V = a * V + oma * d[t]
        if first < 0 and V >= np.float32(THRESHOLD):
            first = t
    return first


# ------------------------------------------------------------------- kernel
def kernel(x, w3, b3, w5, b5, w9, b9, w_red, b_red,
           latency_scale, output_gates, bias, W1, b1, W2, b2):
    x = np.asarray(x, np.float32)
    g64, beta64 = _compose_g(w3, b3, w5, b5, w9, b9, w_red, b_red)
    assert np.abs(beta64).max() < 1e-30, "nonzero conv biases not supported"
    A1, A2, _B1m, _B2m = _build_weights(g64)

    # DR pair slabs: per window, [P*256 + o*128 + c] = A[2P+o][s, c]; tap-8
    # plain slab at 1024.
    AAf = np.zeros((128, AAW), np.float64)
    for ci, Aw in enumerate((A1, A2)):
        base = ci * 1152
        for P in range(4):
            AAf[:, base + P * 256:base + P * 256 + 128] = Aw[2 * P]
            AAf[:, base + P * 256 + 128:base + (P + 1) * 256] = Aw[2 * P + 1]
        AAf[:, base + 1024:base + 1152] = Aw[8]

    x_f8 = x.astype(F8)
    xpad = np.zeros((B, C, PADT), F8)
    xpad[:, :, PAD:PAD + T] = x_f8
    blob = np.zeros((B, 128, BLOBW), F8)
    blob[:, :, 0:AAW] = AAf.astype(F8)[None]
    blob[:, :, XB0:XB0 + 1546] = xpad[:, 0:128, 0:1546]
    blob[:, :, XB1:XB1 + 1546] = xpad[:, 128:256, 0:1546]
    blob[:, :, XB2:XB2 + 2576] = xpad[:, 0:128, 1528:4104]
    blob[:, :, XB3:XB3 + 2576] = xpad[:, 128:256, 1528:4104]

    in_maps = [dict(blob=np.ascontiguousarray(blob[i])) for i in range(NCORES)]

    nc = _get_program()
    res = run_bass_kernel_spmd(nc, in_maps, core_ids=list(range(NCORES)))
    global LAST_RESULTS
    LAST_RESULTS = res

    # smooth-max bound: log(sum_t exp(LAM*V)) / LAM >= Vmax (per row)
    vmax = np.empty((B, C), np.float32)
    for i in range(NCORES):
        es = np.asarray(res.results[i]["esum"], np.float64)
        s1 = es[:, 0:4].sum(axis=1) + es[:, 8]
        s2 = es[:, 4:8].sum(axis=1) + es[:, 9]
        bound = (np.log(np.maximum(s1, 1e-300)) / LAM,
                 np.log(np.maximum(s2, 1e-300)) / LAM)
        vmax[i, PERM1] = bound[0].astype(np.float32)
        vmax[i, PERM2] = bound[1].astype(np.float32)

    global LAST_VMAX
    LAST_VMAX = vmax

    # latency from the Vmax bound; exact host recompute for near-threshold
    # rows (none for the reference input distribution).
    lat = np.full((B, C), np.float32(T), np.float32)
    g32 = g64.astype(np.float32)
    risky = np.argwhere(vmax >= np.float32(FALLBACK_THR))
    for b_, c_ in risky:
        if c_ in STRAD:
            continue  # handled exactly below
        srcs = [(3 * c_ + j) % 256 for j in range(3)]
        first = _exact_row(x[b_, srcs, :], g32[c_], float(beta64[c_]))
        lat[b_, c_] = np.float32(first if first >= 0 else T)

    # straddler channels: the device omits their out-of-window tap; compute
    # them exactly on the host (vectorized over batch).
    nst = len(STRAD)
    dstr = np.zeros((B, nst, T), np.float32)
    for i, c_ in enumerate(STRAD):
        srcs = [(3 * c_ + j) % 256 for j in range(3)]
        xp = np.pad(x[:, srcs, :], ((0, 0), (0, 0), (PAD, PAD)))
        for j in range(3):
            for k in range(9):
                dstr[:, i] += g32[c_, j, k] * xp[:, j, k:k + T]
    a32 = np.float32(ALPHA)
    o32 = np.float32(OMA)
    Vs = np.zeros((B, nst), np.float32)
    first_t = np.full((B, nst), T, np.int64)
    for t in range(T):
        Vs = a32 * Vs + o32 * dstr[:, :, t]
        hit = (Vs >= np.float32(THRESHOLD)) & (first_t == T)
        if hit.any():
            first_t[hit] = t
    for i, c_ in enumerate(STRAD):
        lat[:, c_] = first_t[:, i].astype(np.float32)

    # tiny MLP head (fp32, mirrors reference ops)
    scale = np.maximum(np.asarray(latency_scale, np.float32), np.float32(0.001))
    act = np.exp(-lat / scale).astype(np.float32)
    mixed = (act @ np.asarray(output_gates, np.float32).T
             + np.asarray(bias, np.float32)[None, :]).astype(np.float32)
    h = np.maximum(mixed @ np.asarray(W1, np.float32)
                   + np.asarray(b1, np.float32), np.float32(0)).astype(np.float32)
    raw = (h @ np.asarray(W2, np.float32)
           + np.asarray(b2, np.float32)).astype(np.float32)
    pred = np.clip(np.logaddexp(raw, np.float32(0)), np.float32(0),
                   np.float32(T)).astype(np.float32)
    return pred, lat, act


# revision 12
# speedup vs baseline: 1.0615x; 1.0615x over previous
"""Trainium2 Bass kernel for nn_ChannelLatencySeq2Value (B=8, C=256, T=4096).

Structure (see kernel_baseline.py for the full derivation):
  * encoder collapses to drive[b,c,t] = sum_{j<3,k<9} g[c,j,k] x[b,(3c+j)%256,t+k-4]
  * fp8(e4m3) DoubleRow matmuls pair the taps: per (time-chunk, window):
    4 DR matmuls (taps 0-7, rhs pair dim = stride-1 overlapping shifts) +
    1 plain fp8 matmul (tap 8).  The 4 channels whose 3 input rows straddle
    a 128-row window get their latency computed exactly on the host instead.
  * LIF scan V = a*V + drive via tensor_tensor_scan on Vector, reading the
    PSUM bank directly (no evacuation copy); V stored bf16.
  * per-row Vmax upper bound via exp-sum on the otherwise-idle ScalarE
    (activation Exp with accum_out); host takes log(sum)/LAM >= Vmax.
  * junk warm-up matmuls at the head keep the PE HAM at 2.4 GHz so the real
    stream runs warm; DMA pieces trigger in consumption-priority order.
  * rows whose Vmax bound >= 0.90 are recomputed exactly on the host
    (reference input peaks at ~0.78 incl. smooth-max slack, fp8 device error
    <= ~0.03, so the fallback never triggers but guarantees exactness --
    the bound always over-estimates, so a true crossing can never be missed).
  * tiny (B,C) MLP head on the host in fp32.
"""

import numpy as np
import ml_dtypes

import concourse.bass as bass
import concourse.bacc as bacc
import concourse.mybir as mybir
from concourse.tile import TileContext
from concourse.ap import AP
from concourse.bass_utils import run_bass_kernel_spmd


def _ensure_axon_hooks():
    try:
        import antenv.axon_hooks  # noqa: F401
    except ImportError:
        import sys
        import types
        m = types.ModuleType("antenv.axon_hooks")
        m.get_axon_ntff_profile_hook = lambda: None
        m.set_axon_ntff_profile_hook = lambda h: None
        sys.modules["antenv.axon_hooks"] = m


_ensure_axon_hooks()

# ---------------------------------------------------------------- constants
B, C, T = 8, 256, 4096
OP = 6
ALPHA = float(np.exp(-1.0 / 5.0))
OMA = 1.0 - ALPHA
THRESHOLD = 1.0
TC = 512                      # time chunk (= one PSUM bank of fp32)
NT = T // TC
PAD = 4                       # conv halo (kernel width 9)
PADT = T + 2 * PAD
NCORES = 8
FALLBACK_THR = 0.90           # host exact-recompute margin for Vmax
NWARM = 6                     # junk matmuls that pre-warm the PE HAM clock

F8 = ml_dtypes.float8_e4m3    # == concourse float8e4 (TRN FP8_EXP4)

# mega-blob layout (fp8 bytes per partition, 128 partitions)
# AA: per window: 4 DoubleRow pair slabs (256 B: tap 2P | tap 2P+1) + tap-8
# plain slab (128 B) = 1152 B
AAW = 2 * 1152                 # [0, 2304)
XB0 = AAW                      # x ci0 piece0 (xpad cols [0,1546))
XB1 = XB0 + 1546               # x ci1 piece0
XB2 = XB1 + 1546               # x ci0 piece1 (xpad cols [1528,4104))
XB3 = XB2 + 2576               # x ci1 piece1
BLOBW = XB3 + 2576

# channel -> psum-tile assignment (see kernel_baseline.py)
PERM1 = list(range(0, 42)) + list(range(86, 128)) + list(range(171, 213)) + [42, 85]
PERM2 = list(range(43, 85)) + list(range(128, 170)) + list(range(214, 256)) + [170, 213]
W3ROWS = (0, 127, 128, 255)
STRAD = (42, 85, 170, 213)     # straddler channels: exact host path
LAM = 60.0                     # exp-sum smoothmax sharpness


def _compose_g(w3, b3, w5, b5, w9, b9, w_red, b_red):
    """Collapse the 4-conv encoder into g[c,3,9] (fp64 accum) + beta[c]."""
    g = np.zeros((C, 3, 9), np.float64)
    beta = np.zeros((C,), np.float64)
    paths = [(np.asarray(w3, np.float64), np.asarray(b3, np.float64), 3),
             (np.asarray(w5, np.float64), np.asarray(b5, np.float64), 5),
             (np.asarray(w9, np.float64), np.asarray(b9, np.float64), 9)]
    wr = np.asarray(w_red, np.float64)
    for c in range(C):
        beta[c] += float(b_red[c])
        for i in range(18):
            m = c * 18 + i
            wp, bp, K = paths[m // (C * OP)]
            q = m % (C * OP)
            s = q // OP
            j = (s - 3 * c) % 256
            assert j in (0, 1, 2)
            pad = (K - 1) // 2
            w = wr[c, i, 0]
            beta[c] += w * bp[q]
            g[c, j, 4 - pad:4 + pad + 1] += w * wp[q, 0, :]
    return g, beta


def _build_weights(g):
    """Split (1-a)*g into window lhsT stacks A1/A2 (9,128,128) and straddler
    lhsT B1/B2 (36,128)."""
    gs = g * OMA
    A = [np.zeros((9, 128, 128), np.float64) for _ in range(2)]
    Bm = [np.zeros((36, 128), np.float64) for _ in range(2)]
    for ti, perm in enumerate((PERM1, PERM2)):
        lo = 128 * ti
        for p, c in enumerate(perm):
            for j in range(3):
                s = (3 * c + j) % 256
                if lo <= s < lo + 128:
                    A[ti][:, s - lo, p] = gs[c, j, :]
                else:
                    r = W3ROWS.index(s)
                    Bm[ti][9 * r:9 * r + 9, p] = gs[c, j, :]
    return A[0], A[1], Bm[0], Bm[1]


# ------------------------------------------------------------ device program
_PROG = None
LAST_RESULTS = None
LAST_VMAX = None


def _build_program():
    f32 = mybir.dt.float32
    bf = mybir.dt.bfloat16
    f8 = mybir.dt.float8e4
    DR = mybir.MatmulPerfMode.DoubleRow
    nc = bacc.Bacc(None, target_bir_lowering=False)
    blob_d = nc.declare_dram_parameter("blob", [128, BLOBW], f8, isOutput=False)
    esum_d = nc.declare_dram_parameter("esum", [128, 10], f32, isOutput=True)

    with TileContext(nc) as tc:
        with (
            tc.tile_pool(name="cst", bufs=1) as cst,
            tc.tile_pool(name="ps", bufs=6, space="PSUM") as pp,
            tc.tile_pool(name="pw", bufs=1, space="PSUM") as pw,
            tc.tile_pool(name="dp", bufs=3) as dp,
        ):
            mt = cst.tile([128, BLOBW], f8, tag="mt")
            alpha_t = cst.tile([128, TC], bf, tag="alpha")
            junk = cst.tile([128, TC], bf, tag="junk")
            vb1 = cst.tile([128, T], bf, tag="vb1")
            vb2 = cst.tile([128, T], bf, tag="vb2")
            esc = cst.tile([128, 2 * TC], bf, tag="esc")
            esum_t = cst.tile([128, 10], f32, tag="esum")

            nc.gpsimd.memset(junk[:], 0.0)
            nc.vector.memset(alpha_t[:], ALPHA)

            # PE pre-warm: junk matmuls issue as soon as the memset lands and
            # keep the PE busy through the HAM SHORT window, so the real
            # stream below runs at 2.4 GHz early.
            wps = pw.tile([128, TC], f32, tag="warm")
            for _ in range(NWARM):
                nc.tensor.matmul(wps[:], junk[:, 0:128], junk[:],
                                 start=True, stop=True, skip_group_check=True)

            # loads via SWDGE (gpsimd), in consumption-priority order so the
            # head pieces never share bandwidth with the bulk.
            pieces = [(0, 512, 128),              # ci0 DR slabs P0-P1
                      (XB0, XB0 + 520, 128),      # x ci0 chunk0
                      (512, AAW, 128),            # weight slabs rest
                      (XB1, XB1 + 520, 128),      # x ci1 chunk0
                      (XB0 + 520, XB1, 128),      # x ci0 p0 rest
                      (XB1 + 520, XB2, 128),      # x ci1 p0 rest
                      (XB2, BLOBW, 128)]          # x p1 halves (both ci)
            for lo, hi, np_ in pieces:
                nc.gpsimd.dma_start(out=mt[0:np_, lo:hi], in_=blob_d[0:np_, lo:hi])

            mtt = mt[:].tensor

            # encoder matmuls + LIF scan + running max
            for l in range(NT):
                t0 = l * TC
                for ci, vb in enumerate((vb1, vb2)):
                    if l <= 2:
                        xoff = (XB0, XB1)[ci] + t0
                    else:
                        xoff = (XB2, XB3)[ci] + t0 - 1528
                    aoff = ci * 1152
                    ps = pp.tile([128, TC], f32, tag="ps")
                    for P in range(4):
                        lhsT = AP(mtt, aoff + P * 256,
                                  [[BLOBW, 128], [128, 2], [1, 128]])
                        rhs = AP(mtt, xoff + 2 * P,
                                 [[BLOBW, 128], [1, 2], [1, TC]])
                        nc.tensor.matmul(ps[:], lhsT, rhs, start=(P == 0),
                                         stop=False, perf_mode=DR)
                    nc.tensor.matmul(
                        ps[:], mt[:, aoff + 1024:aoff + 1152],
                        mt[:, xoff + 8:xoff + 8 + TC],
                        start=False, stop=True,
                    )
                    # the scan reads the PSUM bank directly (no evacuation
                    # copy -- ScalarE only runs the exp-sum below)
                    init = 0.0 if l == 0 else vb[:, t0 - 1:t0]
                    nc.vector.tensor_tensor_scan(
                        vb[:, t0:t0 + TC], alpha_t[:], ps[:], init,
                        mybir.AluOpType.mult, mybir.AluOpType.add,
                    )
                    # Vmax upper bound via exp-sum on the otherwise-idle
                    # ScalarE: esum[p] = sum_t exp(LAM*V) over 2-chunk pairs;
                    # host takes log(sum)/LAM >= Vmax.  The final pair is
                    # split per-chunk so the last exp only covers one chunk
                    # (shorter critical tail); its two halves go to separate
                    # esum columns summed on the host.
                    col = 4 * ci + l // 2
                    if l == NT - 2:
                        nc.scalar.activation(
                            out=esc[:, 0:TC], in_=vb[:, t0:t0 + TC],
                            func=mybir.ActivationFunctionType.Exp, scale=LAM,
                            accum_out=esum_t[:, col:col + 1],
                        )
                    elif l == NT - 1:
                        nc.scalar.activation(
                            out=esc[:, TC:2 * TC], in_=vb[:, t0:t0 + TC],
                            func=mybir.ActivationFunctionType.Exp, scale=LAM,
                            accum_out=esum_t[:, 8 + ci:8 + ci + 1],
                        )
                    elif l % 2 == 1:
                        nc.scalar.activation(
                            out=esc[:], in_=vb[:, t0 - TC:t0 + TC],
                            func=mybir.ActivationFunctionType.Exp, scale=LAM,
                            accum_out=esum_t[:, col:col + 1],
                        )
            nc.gpsimd.dma_start(out=esum_d[:], in_=esum_t[:])
    nc.compile()
    return nc


def _get_program():
    global _PROG
    if _PROG is None:
        _PROG = _build_program()
    return _PROG


# ------------------------------------------------------- host-side fallback
def _exact_row(x_row3, g_row, beta_c):
    """Exact fp32 drive + sequential LIF scan + first crossing for one (b,c)."""
    xp = np.pad(x_row3.astype(np.float32), ((0, 0), (PAD, PAD)))
    d = np.full((T,), np.float32(beta_c), np.float32)
    for j in range(3):
        for k in range(9):
            d += np.float32(g_row[j, k]) * xp[j, k:k + T]
    a = np.float32(ALPHA)
    oma = np.float32(OMA)
    V = np.float32(0.0)
    first = -1
    for t in range(T):
        V = a * V + oma * d[t]
        if first < 0 and V >= np.float32(THRESHOLD):
            first = t
    return first


# ------------------------------------------------------------------- kernel
def kernel(x, w3, b3, w5, b5, w9, b9, w_red, b_red,
           latency_scale, output_gates, bias, W1, b1, W2, b2):
    x = np.asarray(x, np.float32)
    g64, beta64 = _compose_g(w3, b3, w5, b5, w9, b9, w_red, b_red)
    assert np.abs(beta64).max() < 1e-30, "nonzero conv biases not supported"
    A1, A2, _B1m, _B2m = _build_weights(g64)

    # DR pair slabs: per window, [P*256 + o*128 + c] = A[2P+o][s, c]; tap-8
    # plain slab at 1024.
    AAf = np.zeros((128, AAW), np.float64)
    for ci, Aw in enumerate((A1, A2)):
        base = ci * 1152
        for P in range(4):
            AAf[:, base + P * 256:base + P * 256 + 128] = Aw[2 * P]
            AAf[:, base + P * 256 + 128:base + (P + 1) * 256] = Aw[2 * P + 1]
        AAf[:, base + 1024:base + 1152] = Aw[8]

    x_f8 = x.astype(F8)
    xpad = np.zeros((B, C, PADT), F8)
    xpad[:, :, PAD:PAD + T] = x_f8
    blob = np.zeros((B, 128, BLOBW), F8)
    blob[:, :, 0:AAW] = AAf.astype(F8)[None]
    blob[:, :, XB0:XB0 + 1546] = xpad[:, 0:128, 0:1546]
    blob[:, :, XB1:XB1 + 1546] = xpad[:, 128:256, 0:1546]
    blob[:, :, XB2:XB2 + 2576] = xpad[:, 0:128, 1528:4104]
    blob[:, :, XB3:XB3 + 2576] = xpad[:, 128:256, 1528:4104]

    in_maps = [dict(blob=np.ascontiguousarray(blob[i])) for i in range(NCORES)]

    nc = _get_program()
    res = run_bass_kernel_spmd(nc, in_maps, core_ids=list(range(NCORES)))
    global LAST_RESULTS
    LAST_RESULTS = res

    # smooth-max bound: log(sum_t exp(LAM*V)) / LAM >= Vmax (per row)
    vmax = np.empty((B, C), np.float32)
    for i in range(NCORES):
        es = np.asarray(res.results[i]["esum"], np.float64)
        s1 = es[:, 0:4].sum(axis=1) + es[:, 8]
        s2 = es[:, 4:8].sum(axis=1) + es[:, 9]
        bound = (np.log(np.maximum(s1, 1e-300)) / LAM,
                 np.log(np.maximum(s2, 1e-300)) / LAM)
        vmax[i, PERM1] = bound[0].astype(np.float32)
        vmax[i, PERM2] = bound[1].astype(np.float32)

    global LAST_VMAX
    LAST_VMAX = vmax

    # latency from the Vmax bound; exact host recompute for near-threshold
    # rows (none for the reference input distribution).
    lat = np.full((B, C), np.float32(T), np.float32)
    g32 = g64.astype(np.float32)
    risky = np.argwhere(vmax >= np.float32(FALLBACK_THR))
    for b_, c_ in risky:
        if c_ in STRAD:
            continue  # handled exactly below
        srcs = [(3 * c_ + j) % 256 for j in range(3)]
        first = _exact_row(x[b_, srcs, :], g32[c_], float(beta64[c_]))
        lat[b_, c_] = np.float32(first if first >= 0 else T)

    # straddler channels: the device omits their out-of-window tap; compute
    # them exactly on the host (vectorized over batch).
    nst = len(STRAD)
    dstr = np.zeros((B, nst, T), np.float32)
    for i, c_ in enumerate(STRAD):
        srcs = [(3 * c_ + j) % 256 for j in range(3)]
        xp = np.pad(x[:, srcs, :], ((0, 0), (0, 0), (PAD, PAD)))
        for j in range(3):
            for k in range(9):
                dstr[:, i] += g32[c_, j, k] * xp[:, j, k:k + T]
    a32 = np.float32(ALPHA)
    o32 = np.float32(OMA)
    Vs = np.zeros((B, nst), np.float32)
    first_t = np.full((B, nst), T, np.int64)
    for t in range(T):
        Vs = a32 * Vs + o32 * dstr[:, :, t]
        hit = (Vs >= np.float32(THRESHOLD)) & (first_t == T)
        if hit.any():
            first_t[hit] = t
    for i, c_ in enumerate(STRAD):
        lat[:, c_] = first_t[:, i].astype(np.float32)

    # tiny MLP head (fp32, mirrors reference ops)
    scale = np.maximum(np.asarray(latency_scale, np.float32), np.float32(0.001))
    act = np.exp(-lat / scale).astype(np.float32)
    mixed = (act @ np.asarray(output_gates, np.float32).T
             + np.asarray(bias, np.float32)[None, :]).astype(np.float32)
    h = np.maximum(mixed @ np.asarray(W1, np.float32)
                   + np.asarray(b1, np.float32), np.float32(0)).astype(np.float32)
    raw = (h @ np.asarray(W2, np.float32)
           + np.asarray(b2, np.float32)).astype(np.float32)
    pred = np.clip(np.logaddexp(raw, np.float32(0)), np.float32(0),
                   np.float32(T)).astype(np.float32)
    return pred, lat, act
